# revision 7
# baseline (speedup 1.0000x reference)
"""YOLOv3 detection-decode kernel for 8 Trainium2 NeuronCores.

Data-parallel over batch (16 images -> 2 per core). Per (image, anchor,
scale) the kernel:
  1. DMAs the [85, H*W] channel-major head slice into SBUF, plus duplicate
     raw w/h rows and a [ones; grid_x; grid_y] constant block -> [90, HW].
  2. Runs one in-place tanh pass (sigmoid(x) = 0.5*tanh(x/2) + 0.5; tanh and
     exp share one ACT table set, sigmoid does not).
  3. For each 128-position chunk, a single f32 matmul against a constant
     [90, 85] weight simultaneously transposes to [pos, 85], applies the
     0.5/0.5 sigmoid affine, the stride scaling, adds stride*grid offsets
     (via the ones/gx/gy weight rows), and passes raw w/h through to
     columns 2/3.
  4. Copies PSUM -> SBUF in 6-chunk (510-column) batches, applies
     exp(x + ln(anchor)) in-place on the strided w/h columns, and DMAs the
     [pos, 85] rows to the output.
"""

import math
import os
import sys

import numpy as np

sys.path.insert(0, "/opt/trn_rl_repo")

N_CORES = 8
B_TOTAL = 16
B_LOC = B_TOTAL // N_CORES  # 2

INP_DIM = 608
NC_CLS = 80  # num classes
CH = 85  # 5 + classes
K_ROWS = 90  # 85 data + 2 raw wh + ones + gx + gy

# (H, W, anchors[(w,h)x3]) per scale; strides 8/16/32
_SCALE_DEFS = [
    (76, 76, [(10.0, 13.0), (16.0, 30.0), (33.0, 23.0)]),
    (38, 38, [(30.0, 61.0), (62.0, 45.0), (59.0, 119.0)]),
    (19, 19, [(116.0, 90.0), (156.0, 198.0), (373.0, 326.0)]),
]


def _scales():
    out = []
    off = 0
    for h, w, anchors in _SCALE_DEFS:
        hw = h * w
        stride = INP_DIM // h
        out.append(dict(H=h, W=w, HW=hw, stride=float(stride), anchors=anchors, off=off))
        off += 3 * hw
    return out, off


SCALES, N_ROWS = _scales()  # N_ROWS == 22743


def _make_weight(stride: float) -> np.ndarray:
    """[90, 85] matmul weight: transpose + sigmoid affine + grid/stride."""
    W = np.zeros((K_ROWS, CH), dtype=np.float32)
    for c in range(CH):
        if c in (0, 1):
            W[c, c] = 0.5 * stride
        elif c in (2, 3):
            pass  # raw wh comes from rows 85/86
        else:
            W[c, c] = 0.5
    W[85, 2] = 1.0
    W[86, 3] = 1.0
    # ones row: sigmoid's +0.5 (stride-scaled for x/y)
    W[87, 0] = W[87, 1] = 0.5 * stride
    W[87, 4:] = 0.5
    W[88, 0] = stride  # gx row
    W[89, 1] = stride  # gy row
    return W


def _make_gridones(h: int, w: int) -> np.ndarray:
    """[3, HW]: ones, grid_x, grid_y."""
    go = np.empty((3, h * w), dtype=np.float32)
    go[0] = 1.0
    go[1] = np.tile(np.arange(w, dtype=np.float32), h)
    go[2] = np.repeat(np.arange(h, dtype=np.float32), w)
    return go


def _patch_tile_drain():
    """The kernel-tail drain Tile emits carries one sem-wait per outstanding
    processor; this container's walrus rejects >1 sync wait on a Drain
    (CoreV3 setupSyncWait "Too many sync wait commands"). Split the waits
    across a chain of single-wait drains — same semantics, compiles."""
    import concourse.mybir as mybir
    from concourse import tile as _tile
    from concourse.vector_clock import ScopedClock

    if getattr(_tile.TileContext, "_drain_split_patched", False):
        return

    def _drain_and_barrier(self, tick_clock, wait_clock):
        drain_inst = self.nc.sync.drain()
        wait_clock.add_sem_waits(
            drain_inst.ins, ScopedClock({None: tick_clock.global_clock})
        )
        si = drain_inst.ins.sync_info
        if si is not None and len(si.on_wait) > 1:
            extra = list(si.on_wait[1:])
            del si.on_wait[1:]
            for w in extra:
                d2 = self.nc.sync.drain()
                si2 = d2.ins.sync_info
                if si2 is None:
                    d2.ins.sync_info = mybir.SyncInfo(on_wait=[w], on_update=[])
                else:
                    si2.on_wait.append(w)
        self.nc.all_engine_barrier()
        assert self.sems is not None
        popped = self.nc._tile_sem_poison_stack.pop()
        assert popped is self._sem_poison
        self.nc.clear_and_free_semaphores(list(self.sems.allocated().values()))
        self.nc.all_engine_barrier()

    _tile.TileContext._drain_and_barrier = _drain_and_barrier
    _tile.TileContext._drain_split_patched = True


_WAIT_CAP = 1


def _split_sync_waits(bir_json: bytes) -> bytes:
    """This container's walrus rejects instructions carrying more than one
    sync wait command. Move extra waits onto injected NoOps immediately
    before the instruction on the same engine queue (sequencers execute in
    order, so the combined wait semantics are identical)."""
    import json as _json

    d = _json.loads(bir_json)
    n = 0
    for f in d.get("functions", []):
        for bb in f.get("blocks", []):
            ins_list = bb.get("instructions", [])
            out = []
            for ins in ins_list:
                si = ins.get("sync_info")
                waits = (si or {}).get("on_wait") or []
                if len(waits) > _WAIT_CAP:
                    keep = waits[-_WAIT_CAP:]
                    extra = waits[: -_WAIT_CAP]
                    for i in range(0, len(extra), _WAIT_CAP):
                        n += 1
                        out.append(
                            {
                                "name": f"I-wsplit-{n}",
                                "opcode": "NoOp",
                                "engine": ins["engine"],
                                "ins": [],
                                "outs": [],
                                "bass_nofuse": True,
                                "sync_info": {
                                    "on_wait": extra[i : i + _WAIT_CAP],
                                    "on_update": [],
                                },
                            }
                        )
                    si["on_wait"] = keep
                out.append(ins)
            bb["instructions"] = out
    return _json.dumps(d).encode()


def _patch_compile():
    import concourse.bass_utils as bu

    if getattr(bu, "_wait_split_patched", False):
        return
    orig = bu.compile_bir_kernel

    def compile_bir_kernel_split(bir_json, tmpdir, neff_name="file.neff"):
        return orig(_split_sync_waits(bir_json), tmpdir, neff_name)

    bu.compile_bir_kernel = compile_bir_kernel_split
    bu._wait_split_patched = True
    import concourse.bass2jax as b2j

    b2j.compile_bir_kernel = compile_bir_kernel_split


def _build_program():
    import concourse.bass as bass
    import concourse.mybir as mybir
    from concourse.tile import TileContext

    _patch_tile_drain()
    _patch_compile()

    f32 = mybir.dt.float32
    AF = mybir.ActivationFunctionType

    nc = bass.Bass()

    x_dram = [
        nc.dram_tensor(f"x{s}", [B_LOC, 255, sc["HW"]], f32, kind="ExternalInput")
        for s, sc in enumerate(SCALES)
    ]
    w_dram = [
        nc.dram_tensor(f"w{s}", [K_ROWS, CH], f32, kind="ExternalInput")
        for s in range(3)
    ]
    go_dram = [
        nc.dram_tensor(f"go{s}", [3, SCALES[s]["HW"]], f32, kind="ExternalInput")
        for s in range(3)
    ]
    # ln(anchor) activation biases, one column per (scale, anchor, w|h),
    # replicated down all 128 partitions.
    ln_dram = nc.dram_tensor("lnA", [128, 18], f32, kind="ExternalInput")
    out = nc.dram_tensor("out", [B_LOC, N_ROWS, CH], f32, kind="ExternalOutput")

    GROUP = 6  # transpose chunks per PSUM bank (6*85 = 510 <= 512 f32)

    with TileContext(nc) as tc:
        with (
            tc.tile_pool(name="consts", bufs=1) as cpool,
            tc.tile_pool(name="res", bufs=3) as rpool,
            tc.tile_pool(name="obuf", bufs=3) as opool,
            tc.tile_pool(name="psum", bufs=6, space="PSUM") as ppool,
        ):
            w_sb = []
            for s in range(3):
                wt = cpool.tile([K_ROWS, CH], f32, tag=f"w{s}")
                nc.sync.dma_start(out=wt[:], in_=w_dram[s][:])
                w_sb.append(wt)
            ln_sb = cpool.tile([128, 18], f32, tag="lnA")
            nc.sync.dma_start(out=ln_sb[:], in_=ln_dram[:])

            for b in range(B_LOC):
                for s, sc in enumerate(SCALES):
                    hw = sc["HW"]
                    nfull = hw // 128
                    rem = hw - 128 * nfull
                    nchunk = nfull + (1 if rem else 0)
                    for a in range(3):
                        res = rpool.tile([K_ROWS, hw], f32, tag="res")
                        nc.sync.dma_start(
                            out=res[0:85, :], in_=x_dram[s][b, 85 * a : 85 * a + 85, :]
                        )
                        nc.sync.dma_start(
                            out=res[85:87, :],
                            in_=x_dram[s][b, 85 * a + 2 : 85 * a + 4, :],
                        )
                        nc.sync.dma_start(out=res[87:90, :], in_=go_dram[s][:])

                        # sigmoid via tanh: rows 2,3 get overwritten garbage
                        # (their weight rows are zero); rows 85..89 stay raw.
                        nc.scalar.activation(
                            out=res[0:85, :], in_=res[0:85, :], func=AF.Tanh, scale=0.5
                        )

                        obuf = opool.tile([128, nchunk * CH], f32, tag="obuf")
                        ngroups = math.ceil(nchunk / GROUP)
                        for g in range(ngroups):
                            k0 = g * GROUP
                            k1 = min(k0 + GROUP, nchunk)
                            psum = ppool.tile([128, GROUP * CH], f32, tag="ps")
                            for k in range(k0, k1):
                                m = min(128, hw - 128 * k)
                                nc.tensor.matmul(
                                    psum[0:m, (k - k0) * CH : (k - k0) * CH + CH],
                                    lhsT=res[:, 128 * k : 128 * k + m],
                                    rhs=w_sb[s][:],
                                    start=True,
                                    stop=True,
                                )
                            # PSUM -> SBUF; avoid copying unwritten partitions
                            # of the partial chunk (stale data).
                            n_fullchunks_in_g = min(k1, nfull) - k0
                            if n_fullchunks_in_g > 0:
                                wcols = n_fullchunks_in_g * CH
                                nc.vector.tensor_copy(
                                    out=obuf[:, k0 * CH : k0 * CH + wcols],
                                    in_=psum[:, 0:wcols],
                                )
                            if k1 == nchunk and rem:
                                pcol = (nfull - k0) * CH
                                nc.vector.tensor_copy(
                                    out=obuf[0:rem, nfull * CH : nfull * CH + CH],
                                    in_=psum[0:rem, pcol : pcol + CH],
                                )

                        # w/h: exp(x + ln(anchor)) in place on strided cols 2,3
                        ob3 = obuf.rearrange("p (k c) -> p k c", c=CH)
                        for col in (2, 3):
                            j = (s * 3 + a) * 2 + (col - 2)
                            if nfull:
                                nc.scalar.activation(
                                    out=ob3[:, 0:nfull, col : col + 1],
                                    in_=ob3[:, 0:nfull, col : col + 1],
                                    func=AF.Exp,
                                    bias=ln_sb[:, j : j + 1],
                                )
                            if rem:
                                nc.scalar.activation(
                                    out=ob3[0:rem, nfull : nfull + 1, col : col + 1],
                                    in_=ob3[0:rem, nfull : nfull + 1, col : col + 1],
                                    func=AF.Exp,
                                    bias=ln_sb[0:rem, j : j + 1],
                                )

                        base = sc["off"] + a * hw
                        if nfull:
                            src = obuf[:, 0 : nfull * CH].rearrange(
                                "p (k c) -> p k c", c=CH
                            )
                            dst = out[b, base : base + 128 * nfull, :].rearrange(
                                "(k p) c -> p k c", p=128
                            )
                            nc.sync.dma_start(out=dst, in_=src)
                        if rem:
                            nc.sync.dma_start(
                                out=out[b, base + 128 * nfull : base + hw, :],
                                in_=obuf[0:rem, nfull * CH : nfull * CH + CH],
                            )
    return nc


_PROGRAM = None
LAST_RESULT = None


def _get_program():
    global _PROGRAM
    if _PROGRAM is None:
        _PROGRAM = _build_program()
    return _PROGRAM


def kernel(x1: np.ndarray, x2: np.ndarray, x3: np.ndarray) -> np.ndarray:
    global LAST_RESULT
    from concourse.bass_utils import run_bass_kernel_spmd

    nc = _get_program()

    xs = [
        np.ascontiguousarray(x, dtype=np.float32).reshape(B_TOTAL, 255, sc["HW"])
        for x, sc in zip((x1, x2, x3), SCALES)
    ]
    w_consts = [_make_weight(sc["stride"]) for sc in SCALES]
    go_consts = [_make_gridones(sc["H"], sc["W"]) for sc in SCALES]
    ln_vals = np.array(
        [math.log(v) for sc in SCALES for anc in sc["anchors"] for v in anc],
        dtype=np.float32,
    )
    ln_const = np.broadcast_to(ln_vals, (128, 18)).copy()

    in_maps = []
    for i in range(N_CORES):
        m = {"lnA": ln_const}
        for s in range(3):
            m[f"x{s}"] = xs[s][i * B_LOC : (i + 1) * B_LOC]
            m[f"w{s}"] = w_consts[s]
            m[f"go{s}"] = go_consts[s]
        in_maps.append(m)

    LAST_RESULT = run_bass_kernel_spmd(nc, in_maps, core_ids=list(range(N_CORES)))
    return np.concatenate([r["out"] for r in LAST_RESULT.results], axis=0)


# revision 12
# speedup vs baseline: 1.2303x; 1.2303x over previous
"""YOLOv3 detection-decode kernel for 8 Trainium2 NeuronCores.

Data-parallel over batch (16 images -> 2 per core). Per (image, anchor,
scale) the kernel:
  1. DMAs the [85, H*W] channel-major head slice into SBUF, plus duplicate
     raw w/h rows and a [ones; grid_x; grid_y] constant block -> [90, HW].
  2. Runs one in-place tanh pass (sigmoid(x) = 0.5*tanh(x/2) + 0.5; tanh and
     exp share one ACT table set, sigmoid does not).
  3. For each 128-position chunk, a single f32 matmul against a constant
     [90, 85] weight simultaneously transposes to [pos, 85], applies the
     0.5/0.5 sigmoid affine, the stride scaling, adds stride*grid offsets
     (via the ones/gx/gy weight rows), and passes raw w/h through to
     columns 2/3.
  4. Copies PSUM -> SBUF in 6-chunk (510-column) batches, applies
     exp(x + ln(anchor)) in-place on the strided w/h columns, and DMAs the
     [pos, 85] rows to the output.
"""

import math
import os
import sys

import numpy as np

sys.path.insert(0, "/opt/trn_rl_repo")

N_CORES = 8
B_TOTAL = 16
B_LOC = B_TOTAL // N_CORES  # 2

INP_DIM = 608
NC_CLS = 80  # num classes
CH = 85  # 5 + classes
K_ROWS = 90  # 85 data + 2 raw wh + ones + gx + gy

# (H, W, anchors[(w,h)x3]) per scale; strides 8/16/32
_SCALE_DEFS = [
    (76, 76, [(10.0, 13.0), (16.0, 30.0), (33.0, 23.0)]),
    (38, 38, [(30.0, 61.0), (62.0, 45.0), (59.0, 119.0)]),
    (19, 19, [(116.0, 90.0), (156.0, 198.0), (373.0, 326.0)]),
]


def _scales():
    out = []
    off = 0
    for h, w, anchors in _SCALE_DEFS:
        hw = h * w
        stride = INP_DIM // h
        out.append(dict(H=h, W=w, HW=hw, stride=float(stride), anchors=anchors, off=off))
        off += 3 * hw
    return out, off


SCALES, N_ROWS = _scales()  # N_ROWS == 22743


def _make_weight(stride: float) -> np.ndarray:
    """[90, 85] matmul weight: transpose + sigmoid affine + grid/stride.
    All entries (0.5, 0.5*stride, stride, 1) are exact in bf16."""
    W = np.zeros((K_ROWS, CH), dtype=np.float32)
    for c in range(CH):
        if c in (0, 1):
            W[c, c] = 0.5 * stride
        elif c in (2, 3):
            pass  # raw wh comes from rows 85/86
        else:
            W[c, c] = 0.5
    W[85, 2] = 1.0
    W[86, 3] = 1.0
    # ones row: sigmoid's +0.5 (stride-scaled for x/y)
    W[87, 0] = W[87, 1] = 0.5 * stride
    W[87, 4:] = 0.5
    W[88, 0] = stride  # gx row
    W[89, 1] = stride  # gy row
    return W


def _make_gridones(h: int, w: int) -> np.ndarray:
    """[3, HW]: ones, grid_x, grid_y."""
    go = np.empty((3, h * w), dtype=np.float32)
    go[0] = 1.0
    go[1] = np.tile(np.arange(w, dtype=np.float32), h)
    go[2] = np.repeat(np.arange(h, dtype=np.float32), w)
    return go


def _patch_tile_drain():
    """The kernel-tail drain Tile emits carries one sem-wait per outstanding
    processor; this container's walrus rejects >1 sync wait on a Drain
    (CoreV3 setupSyncWait "Too many sync wait commands"). Split the waits
    across a chain of single-wait drains — same semantics, compiles."""
    import concourse.mybir as mybir
    from concourse import tile as _tile
    from concourse.vector_clock import ScopedClock

    if getattr(_tile.TileContext, "_drain_split_patched", False):
        return

    def _drain_and_barrier(self, tick_clock, wait_clock):
        drain_inst = self.nc.sync.drain()
        wait_clock.add_sem_waits(
            drain_inst.ins, ScopedClock({None: tick_clock.global_clock})
        )
        si = drain_inst.ins.sync_info
        if si is not None and len(si.on_wait) > 1:
            extra = list(si.on_wait[1:])
            del si.on_wait[1:]
            for w in extra:
                d2 = self.nc.sync.drain()
                si2 = d2.ins.sync_info
                if si2 is None:
                    d2.ins.sync_info = mybir.SyncInfo(on_wait=[w], on_update=[])
                else:
                    si2.on_wait.append(w)
        self.nc.all_engine_barrier()
        assert self.sems is not None
        popped = self.nc._tile_sem_poison_stack.pop()
        assert popped is self._sem_poison
        self.nc.clear_and_free_semaphores(list(self.sems.allocated().values()))
        self.nc.all_engine_barrier()

    _tile.TileContext._drain_and_barrier = _drain_and_barrier
    _tile.TileContext._drain_split_patched = True


_WAIT_CAP = 1


def _split_sync_waits(bir_json: bytes) -> bytes:
    """This container's walrus rejects instructions carrying more than one
    sync wait command. Move extra waits onto injected NoOps immediately
    before the instruction on the same engine queue (sequencers execute in
    order, so the combined wait semantics are identical)."""
    import json as _json

    d = _json.loads(bir_json)
    n = 0
    for f in d.get("functions", []):
        for bb in f.get("blocks", []):
            ins_list = bb.get("instructions", [])
            out = []
            for ins in ins_list:
                si = ins.get("sync_info")
                waits = (si or {}).get("on_wait") or []
                if len(waits) > _WAIT_CAP:
                    keep = waits[-_WAIT_CAP:]
                    extra = waits[: -_WAIT_CAP]
                    for i in range(0, len(extra), _WAIT_CAP):
                        n += 1
                        out.append(
                            {
                                "name": f"I-wsplit-{n}",
                                "opcode": "NoOp",
                                "engine": ins["engine"],
                                "ins": [],
                                "outs": [],
                                "bass_nofuse": True,
                                "sync_info": {
                                    "on_wait": extra[i : i + _WAIT_CAP],
                                    "on_update": [],
                                },
                            }
                        )
                    si["on_wait"] = keep
                out.append(ins)
            bb["instructions"] = out
    return _json.dumps(d).encode()


def _patch_compile():
    import concourse.bass_utils as bu

    if getattr(bu, "_wait_split_patched", False):
        return
    orig = bu.compile_bir_kernel

    def compile_bir_kernel_split(bir_json, tmpdir, neff_name="file.neff"):
        return orig(_split_sync_waits(bir_json), tmpdir, neff_name)

    bu.compile_bir_kernel = compile_bir_kernel_split
    bu._wait_split_patched = True
    import concourse.bass2jax as b2j

    b2j.compile_bir_kernel = compile_bir_kernel_split


def _build_program():
    import concourse.bass as bass
    import concourse.mybir as mybir
    from concourse.tile import TileContext

    _patch_tile_drain()
    _patch_compile()

    f32 = mybir.dt.float32
    bf16 = mybir.dt.bfloat16
    AF = mybir.ActivationFunctionType

    nc = bass.Bass()

    x_dram = [
        nc.dram_tensor(f"x{s}", [B_LOC, 255, sc["HW"]], f32, kind="ExternalInput")
        for s, sc in enumerate(SCALES)
    ]
    w_dram = [
        nc.dram_tensor(f"w{s}", [K_ROWS, CH], bf16, kind="ExternalInput")
        for s in range(3)
    ]
    go_dram = [
        nc.dram_tensor(f"go{s}", [3, SCALES[s]["HW"]], bf16, kind="ExternalInput")
        for s in range(3)
    ]
    # ln(anchor) activation biases, one column per (scale, anchor, w|h),
    # replicated down all 128 partitions.
    ln_dram = nc.dram_tensor("lnA", [128, 18], f32, kind="ExternalInput")
    out = nc.dram_tensor("out", [B_LOC, N_ROWS, CH], f32, kind="ExternalOutput")

    GROUP = 6  # transpose chunks per PSUM bank (6*85 = 510 <= 512 f32)

    with TileContext(nc) as tc:
        with (
            tc.tile_pool(name="consts", bufs=1) as cpool,
            tc.tile_pool(name="res", bufs=3) as rpool,
            tc.tile_pool(name="obuf", bufs=3) as opool,
            tc.tile_pool(name="psum", bufs=6, space="PSUM") as ppool,
        ):
            w_sb = []
            for s in range(3):
                wt = cpool.tile([K_ROWS, CH], bf16, tag=f"w{s}")
                nc.sync.dma_start(out=wt[:], in_=w_dram[s][:])
                w_sb.append(wt)
            ln_sb = cpool.tile([128, 18], f32, tag="lnA")
            nc.sync.dma_start(out=ln_sb[:], in_=ln_dram[:])

            for b in range(B_LOC):
                for s, sc in enumerate(SCALES):
                    hw = sc["HW"]
                    nfull = hw // 128
                    rem = hw - 128 * nfull
                    nchunk = nfull + (1 if rem else 0)
                    for a in range(3):
                        # raw f32 head slice; tanh casts it into the bf16
                        # matmul operand tile.
                        xf = rpool.tile([85, hw], f32, tag="xf")
                        nc.sync.dma_start(
                            out=xf[:], in_=x_dram[s][b, 85 * a : 85 * a + 85, :]
                        )
                        res = rpool.tile([K_ROWS, hw], bf16, tag="res")
                        # raw w/h rows, cast f32->bf16 during DMA (SWDGE)
                        nc.gpsimd.dma_start(
                            out=res[85:87, :],
                            in_=x_dram[s][b, 85 * a + 2 : 85 * a + 4, :],
                        )
                        nc.sync.dma_start(out=res[87:90, :], in_=go_dram[s][:])

                        # sigmoid via tanh: rows 2,3 get overwritten garbage
                        # (their weight rows are zero); rows 85..89 stay raw.
                        nc.scalar.activation(
                            out=res[0:85, :], in_=xf[:], func=AF.Tanh, scale=0.5
                        )

                        obuf = opool.tile([128, nchunk * CH], f32, tag="obuf")
                        ngroups = math.ceil(nchunk / GROUP)
                        for g in range(ngroups):
                            k0 = g * GROUP
                            k1 = min(k0 + GROUP, nchunk)
                            psum = ppool.tile([128, GROUP * CH], f32, tag="ps")
                            for k in range(k0, k1):
                                m = min(128, hw - 128 * k)
                                nc.tensor.matmul(
                                    psum[0:m, (k - k0) * CH : (k - k0) * CH + CH],
                                    lhsT=res[:, 128 * k : 128 * k + m],
                                    rhs=w_sb[s][:],
                                    start=True,
                                    stop=True,
                                )
                            # PSUM -> SBUF; avoid copying unwritten partitions
                            # of the partial chunk (stale data).
                            n_fullchunks_in_g = min(k1, nfull) - k0
                            if n_fullchunks_in_g > 0:
                                wcols = n_fullchunks_in_g * CH
                                nc.vector.tensor_copy(
                                    out=obuf[:, k0 * CH : k0 * CH + wcols],
                                    in_=psum[:, 0:wcols],
                                )
                            if k1 == nchunk and rem:
                                pcol = (nfull - k0) * CH
                                nc.vector.tensor_copy(
                                    out=obuf[0:rem, nfull * CH : nfull * CH + CH],
                                    in_=psum[0:rem, pcol : pcol + CH],
                                )

                        # w/h: exp(x + ln(anchor)) in place on strided cols 2,3
                        ob3 = obuf.rearrange("p (k c) -> p k c", c=CH)
                        for col in (2, 3):
                            j = (s * 3 + a) * 2 + (col - 2)
                            if nfull:
                                nc.scalar.activation(
                                    out=ob3[:, 0:nfull, col : col + 1],
                                    in_=ob3[:, 0:nfull, col : col + 1],
                                    func=AF.Exp,
                                    bias=ln_sb[:, j : j + 1],
                                )
                            if rem:
                                nc.scalar.activation(
                                    out=ob3[0:rem, nfull : nfull + 1, col : col + 1],
                                    in_=ob3[0:rem, nfull : nfull + 1, col : col + 1],
                                    func=AF.Exp,
                                    bias=ln_sb[0:rem, j : j + 1],
                                )

                        base = sc["off"] + a * hw
                        if nfull:
                            src = obuf[:, 0 : nfull * CH].rearrange(
                                "p (k c) -> p k c", c=CH
                            )
                            dst = out[b, base : base + 128 * nfull, :].rearrange(
                                "(k p) c -> p k c", p=128
                            )
                            nc.sync.dma_start(out=dst, in_=src)
                        if rem:
                            nc.sync.dma_start(
                                out=out[b, base + 128 * nfull : base + hw, :],
                                in_=obuf[0:rem, nfull * CH : nfull * CH + CH],
                            )
    return nc


_PROGRAM = None
LAST_RESULT = None


def _get_program():
    global _PROGRAM
    if _PROGRAM is None:
        _PROGRAM = _build_program()
    return _PROGRAM


def kernel(x1: np.ndarray, x2: np.ndarray, x3: np.ndarray) -> np.ndarray:
    global LAST_RESULT
    from concourse.bass_utils import run_bass_kernel_spmd

    nc = _get_program()

    import ml_dtypes

    bf16 = ml_dtypes.bfloat16
    xs = [
        np.ascontiguousarray(x, dtype=np.float32).reshape(B_TOTAL, 255, sc["HW"])
        for x, sc in zip((x1, x2, x3), SCALES)
    ]
    w_consts = [_make_weight(sc["stride"]).astype(bf16) for sc in SCALES]
    go_consts = [_make_gridones(sc["H"], sc["W"]).astype(bf16) for sc in SCALES]
    ln_vals = np.array(
        [math.log(v) for sc in SCALES for anc in sc["anchors"] for v in anc],
        dtype=np.float32,
    )
    ln_const = np.broadcast_to(ln_vals, (128, 18)).copy()

    in_maps = []
    for i in range(N_CORES):
        m = {"lnA": ln_const}
        for s in range(3):
            m[f"x{s}"] = xs[s][i * B_LOC : (i + 1) * B_LOC]
            m[f"w{s}"] = w_consts[s]
            m[f"go{s}"] = go_consts[s]
        in_maps.append(m)

    LAST_RESULT = run_bass_kernel_spmd(nc, in_maps, core_ids=list(range(N_CORES)))
    return np.concatenate([r["out"] for r in LAST_RESULT.results], axis=0)


# revision 14
# speedup vs baseline: 1.9769x; 1.6069x over previous
"""YOLOv3 detection-decode kernel for 8 Trainium2 NeuronCores.

Data-parallel over batch (16 images -> 2 per core). Per (image, anchor,
scale) the kernel:
  1. DMAs the [85, H*W] channel-major head slice into SBUF, plus duplicate
     raw w/h rows and a [ones; grid_x; grid_y] constant block -> [90, HW].
  2. Runs one in-place tanh pass (sigmoid(x) = 0.5*tanh(x/2) + 0.5; tanh and
     exp share one ACT table set, sigmoid does not).
  3. For each 128-position chunk, a single f32 matmul against a constant
     [90, 85] weight simultaneously transposes to [pos, 85], applies the
     0.5/0.5 sigmoid affine, the stride scaling, adds stride*grid offsets
     (via the ones/gx/gy weight rows), and passes raw w/h through to
     columns 2/3.
  4. Copies PSUM -> SBUF in 6-chunk (510-column) batches, applies
     exp(x + ln(anchor)) in-place on the strided w/h columns, and DMAs the
     [pos, 85] rows to the output.
"""

import math
import os
import sys

import numpy as np

sys.path.insert(0, "/opt/trn_rl_repo")

N_CORES = 8
B_TOTAL = 16
B_LOC = B_TOTAL // N_CORES  # 2

INP_DIM = 608
NC_CLS = 80  # num classes
CH = 85  # 5 + classes
K_ROWS = 90  # 85 data + 2 raw wh + ones + gx + gy

# (H, W, anchors[(w,h)x3]) per scale; strides 8/16/32
_SCALE_DEFS = [
    (76, 76, [(10.0, 13.0), (16.0, 30.0), (33.0, 23.0)]),
    (38, 38, [(30.0, 61.0), (62.0, 45.0), (59.0, 119.0)]),
    (19, 19, [(116.0, 90.0), (156.0, 198.0), (373.0, 326.0)]),
]


def _scales():
    out = []
    off = 0
    for h, w, anchors in _SCALE_DEFS:
        hw = h * w
        stride = INP_DIM // h
        out.append(dict(H=h, W=w, HW=hw, stride=float(stride), anchors=anchors, off=off))
        off += 3 * hw
    return out, off


SCALES, N_ROWS = _scales()  # N_ROWS == 22743


def _make_weight(stride: float) -> np.ndarray:
    """[90, 85] matmul weight: transpose + sigmoid affine + grid/stride.
    All entries (0.5, 0.5*stride, stride, 1) are exact in bf16."""
    W = np.zeros((K_ROWS, CH), dtype=np.float32)
    for c in range(CH):
        if c in (0, 1):
            W[c, c] = 0.5 * stride
        elif c in (2, 3):
            pass  # raw wh comes from rows 85/86
        else:
            W[c, c] = 0.5
    W[85, 2] = 1.0
    W[86, 3] = 1.0
    # ones row: sigmoid's +0.5 (stride-scaled for x/y)
    W[87, 0] = W[87, 1] = 0.5 * stride
    W[87, 4:] = 0.5
    W[88, 0] = stride  # gx row
    W[89, 1] = stride  # gy row
    return W


def _make_gridones(h: int, w: int) -> np.ndarray:
    """[3, HW]: ones, grid_x, grid_y."""
    go = np.empty((3, h * w), dtype=np.float32)
    go[0] = 1.0
    go[1] = np.tile(np.arange(w, dtype=np.float32), h)
    go[2] = np.repeat(np.arange(h, dtype=np.float32), w)
    return go


def _patch_tile_drain():
    """The kernel-tail drain Tile emits carries one sem-wait per outstanding
    processor; this container's walrus rejects >1 sync wait on a Drain
    (CoreV3 setupSyncWait "Too many sync wait commands"). Split the waits
    across a chain of single-wait drains — same semantics, compiles."""
    import concourse.mybir as mybir
    from concourse import tile as _tile
    from concourse.vector_clock import ScopedClock

    if getattr(_tile.TileContext, "_drain_split_patched", False):
        return

    def _drain_and_barrier(self, tick_clock, wait_clock):
        drain_inst = self.nc.sync.drain()
        wait_clock.add_sem_waits(
            drain_inst.ins, ScopedClock({None: tick_clock.global_clock})
        )
        si = drain_inst.ins.sync_info
        if si is not None and len(si.on_wait) > 1:
            extra = list(si.on_wait[1:])
            del si.on_wait[1:]
            for w in extra:
                d2 = self.nc.sync.drain()
                si2 = d2.ins.sync_info
                if si2 is None:
                    d2.ins.sync_info = mybir.SyncInfo(on_wait=[w], on_update=[])
                else:
                    si2.on_wait.append(w)
        self.nc.all_engine_barrier()
        assert self.sems is not None
        popped = self.nc._tile_sem_poison_stack.pop()
        assert popped is self._sem_poison
        self.nc.clear_and_free_semaphores(list(self.sems.allocated().values()))
        self.nc.all_engine_barrier()

    _tile.TileContext._drain_and_barrier = _drain_and_barrier
    _tile.TileContext._drain_split_patched = True


_WAIT_CAP = 1


def _split_sync_waits(bir_json: bytes) -> bytes:
    """This container's walrus rejects instructions carrying more than one
    sync wait command. Move extra waits onto injected NoOps immediately
    before the instruction on the same engine queue (sequencers execute in
    order, so the combined wait semantics are identical)."""
    import json as _json

    d = _json.loads(bir_json)
    n = 0
    for f in d.get("functions", []):
        for bb in f.get("blocks", []):
            ins_list = bb.get("instructions", [])
            out = []
            for ins in ins_list:
                si = ins.get("sync_info")
                waits = (si or {}).get("on_wait") or []
                if len(waits) > _WAIT_CAP:
                    keep = waits[-_WAIT_CAP:]
                    extra = waits[: -_WAIT_CAP]
                    for i in range(0, len(extra), _WAIT_CAP):
                        n += 1
                        out.append(
                            {
                                "name": f"I-wsplit-{n}",
                                "opcode": "NoOp",
                                "engine": ins["engine"],
                                "ins": [],
                                "outs": [],
                                "bass_nofuse": True,
                                "sync_info": {
                                    "on_wait": extra[i : i + _WAIT_CAP],
                                    "on_update": [],
                                },
                            }
                        )
                    si["on_wait"] = keep
                out.append(ins)
            bb["instructions"] = out
    return _json.dumps(d).encode()


def _patch_compile():
    import concourse.bass_utils as bu

    if getattr(bu, "_wait_split_patched", False):
        return
    orig = bu.compile_bir_kernel

    def compile_bir_kernel_split(bir_json, tmpdir, neff_name="file.neff"):
        return orig(_split_sync_waits(bir_json), tmpdir, neff_name)

    bu.compile_bir_kernel = compile_bir_kernel_split
    bu._wait_split_patched = True
    import concourse.bass2jax as b2j

    b2j.compile_bir_kernel = compile_bir_kernel_split


def _build_program():
    import concourse.bass as bass
    import concourse.mybir as mybir
    from concourse.tile import TileContext

    _patch_tile_drain()
    _patch_compile()

    f32 = mybir.dt.float32
    bf16 = mybir.dt.bfloat16
    AF = mybir.ActivationFunctionType

    nc = bass.Bass()

    x_dram = [
        nc.dram_tensor(f"x{s}", [B_LOC, 255, sc["HW"]], f32, kind="ExternalInput")
        for s, sc in enumerate(SCALES)
    ]
    w_dram = [
        nc.dram_tensor(f"w{s}", [K_ROWS, CH], bf16, kind="ExternalInput")
        for s in range(3)
    ]
    go_dram = [
        nc.dram_tensor(f"go{s}", [3, SCALES[s]["HW"]], bf16, kind="ExternalInput")
        for s in range(3)
    ]
    # ln(anchor) activation biases, one column per (scale, anchor, w|h),
    # replicated down all 128 partitions.
    ln_dram = nc.dram_tensor("lnA", [128, 18], f32, kind="ExternalInput")
    out = nc.dram_tensor("out", [B_LOC, N_ROWS, CH], f32, kind="ExternalOutput")

    GROUP = 6  # transpose chunks per PSUM bank (6*85 = 510 <= 512 f32)

    with TileContext(nc) as tc:
        with (
            tc.tile_pool(name="consts", bufs=1) as cpool,
            tc.tile_pool(name="res", bufs=3) as rpool,
            tc.tile_pool(name="obuf", bufs=3) as opool,
            tc.tile_pool(name="psum", bufs=6, space="PSUM") as ppool,
        ):
            w_sb = []
            for s in range(3):
                wt = cpool.tile([K_ROWS, CH], bf16, tag=f"w{s}")
                nc.sync.dma_start(out=wt[:], in_=w_dram[s][:])
                w_sb.append(wt)
            ln_sb = cpool.tile([128, 18], f32, tag="lnA")
            nc.sync.dma_start(out=ln_sb[:], in_=ln_dram[:])

            for b in range(B_LOC):
                for s, sc in enumerate(SCALES):
                    hw = sc["HW"]
                    nfull = hw // 128
                    rem = hw - 128 * nfull
                    nchunk = nfull + (1 if rem else 0)
                    for a in range(3):
                        # raw f32 head slice; tanh casts it into the bf16
                        # matmul operand tile.
                        xf = rpool.tile([85, hw], f32, tag="xf")
                        nc.sync.dma_start(
                            out=xf[:], in_=x_dram[s][b, 85 * a : 85 * a + 85, :]
                        )
                        res = rpool.tile([K_ROWS, hw], bf16, tag="res")
                        # raw w/h rows, cast f32->bf16 during DMA (SWDGE)
                        nc.gpsimd.dma_start(
                            out=res[85:87, :],
                            in_=x_dram[s][b, 85 * a + 2 : 85 * a + 4, :],
                        )
                        nc.sync.dma_start(out=res[87:90, :], in_=go_dram[s][:])

                        # sigmoid via tanh: rows 2,3 get overwritten garbage
                        # (their weight rows are zero); rows 85..89 stay raw.
                        nc.scalar.activation(
                            out=res[0:85, :], in_=xf[:], func=AF.Tanh, scale=0.5
                        )

                        obuf = opool.tile([128, nchunk * CH], f32, tag="obuf")
                        # Strided position chunks: chunk k covers positions
                        # {k + nfull*i}, so PSUM/obuf partition i accumulates
                        # nfull consecutive output rows -> the store DMA gets
                        # nfull*340B contiguous per partition.
                        res_str = (
                            res[:, 0 : nfull * 128].rearrange(
                                "p (i r) -> p r i", r=nfull
                            )
                            if nfull
                            else None
                        )
                        ngroups = math.ceil(nchunk / GROUP)
                        for g in range(ngroups):
                            k0 = g * GROUP
                            k1 = min(k0 + GROUP, nchunk)
                            psum = ppool.tile([128, GROUP * CH], f32, tag="ps")
                            for k in range(k0, k1):
                                if k < nfull:
                                    lhsT = res_str[:, k, :]
                                    m = 128
                                else:
                                    lhsT = res[:, 128 * nfull : hw]
                                    m = rem
                                nc.tensor.matmul(
                                    psum[0:m, (k - k0) * CH : (k - k0) * CH + CH],
                                    lhsT=lhsT,
                                    rhs=w_sb[s][:],
                                    start=True,
                                    stop=True,
                                )
                            # PSUM -> SBUF; avoid copying unwritten partitions
                            # of the partial chunk (stale data).
                            n_fullchunks_in_g = min(k1, nfull) - k0
                            if n_fullchunks_in_g > 0:
                                wcols = n_fullchunks_in_g * CH
                                nc.vector.tensor_copy(
                                    out=obuf[:, k0 * CH : k0 * CH + wcols],
                                    in_=psum[:, 0:wcols],
                                )
                            if k1 == nchunk and rem:
                                pcol = (nfull - k0) * CH
                                nc.vector.tensor_copy(
                                    out=obuf[0:rem, nfull * CH : nfull * CH + CH],
                                    in_=psum[0:rem, pcol : pcol + CH],
                                )

                        # w/h: exp(x + ln(anchor)) in place on strided cols 2,3
                        ob3 = obuf.rearrange("p (k c) -> p k c", c=CH)
                        for col in (2, 3):
                            j = (s * 3 + a) * 2 + (col - 2)
                            if nfull:
                                nc.scalar.activation(
                                    out=ob3[:, 0:nfull, col : col + 1],
                                    in_=ob3[:, 0:nfull, col : col + 1],
                                    func=AF.Exp,
                                    bias=ln_sb[:, j : j + 1],
                                )
                            if rem:
                                nc.scalar.activation(
                                    out=ob3[0:rem, nfull : nfull + 1, col : col + 1],
                                    in_=ob3[0:rem, nfull : nfull + 1, col : col + 1],
                                    func=AF.Exp,
                                    bias=ln_sb[0:rem, j : j + 1],
                                )

                        base = sc["off"] + a * hw
                        if nfull:
                            # partition p <-> rows [base + p*nfull, +nfull):
                            # one contiguous nfull*340B descriptor per partition
                            dst = out[b, base : base + 128 * nfull, :].rearrange(
                                "(p r) c -> p (r c)", p=128
                            )
                            nc.scalar.dma_start(out=dst, in_=obuf[:, 0 : nfull * CH])
                        if rem:
                            nc.scalar.dma_start(
                                out=out[b, base + 128 * nfull : base + hw, :],
                                in_=obuf[0:rem, nfull * CH : nfull * CH + CH],
                            )
    return nc


_PROGRAM = None
LAST_RESULT = None


def _get_program():
    global _PROGRAM
    if _PROGRAM is None:
        _PROGRAM = _build_program()
    return _PROGRAM


def kernel(x1: np.ndarray, x2: np.ndarray, x3: np.ndarray) -> np.ndarray:
    global LAST_RESULT
    from concourse.bass_utils import run_bass_kernel_spmd

    nc = _get_program()

    import ml_dtypes

    bf16 = ml_dtypes.bfloat16
    xs = [
        np.ascontiguousarray(x, dtype=np.float32).reshape(B_TOTAL, 255, sc["HW"])
        for x, sc in zip((x1, x2, x3), SCALES)
    ]
    w_consts = [_make_weight(sc["stride"]).astype(bf16) for sc in SCALES]
    go_consts = [_make_gridones(sc["H"], sc["W"]).astype(bf16) for sc in SCALES]
    ln_vals = np.array(
        [math.log(v) for sc in SCALES for anc in sc["anchors"] for v in anc],
        dtype=np.float32,
    )
    ln_const = np.broadcast_to(ln_vals, (128, 18)).copy()

    in_maps = []
    for i in range(N_CORES):
        m = {"lnA": ln_const}
        for s in range(3):
            m[f"x{s}"] = xs[s][i * B_LOC : (i + 1) * B_LOC]
            m[f"w{s}"] = w_consts[s]
            m[f"go{s}"] = go_consts[s]
        in_maps.append(m)

    LAST_RESULT = run_bass_kernel_spmd(nc, in_maps, core_ids=list(range(N_CORES)))
    return np.concatenate([r["out"] for r in LAST_RESULT.results], axis=0)


# revision 16
# speedup vs baseline: 2.7102x; 1.3709x over previous
"""YOLOv3 detection-decode kernel for 8 Trainium2 NeuronCores.

Data-parallel over batch (16 images -> 2 per core). Per (image, anchor,
scale) the kernel:
  1. DMAs the [85, H*W] channel-major head slice into SBUF, plus duplicate
     raw w/h rows and a [ones; grid_x; grid_y] constant block -> [90, HW].
  2. Runs one in-place tanh pass (sigmoid(x) = 0.5*tanh(x/2) + 0.5; tanh and
     exp share one ACT table set, sigmoid does not).
  3. For each 128-position chunk, a single f32 matmul against a constant
     [90, 85] weight simultaneously transposes to [pos, 85], applies the
     0.5/0.5 sigmoid affine, the stride scaling, adds stride*grid offsets
     (via the ones/gx/gy weight rows), and passes raw w/h through to
     columns 2/3.
  4. Copies PSUM -> SBUF in 6-chunk (510-column) batches, applies
     exp(x + ln(anchor)) in-place on the strided w/h columns, and DMAs the
     [pos, 85] rows to the output.
"""

import math
import os
import sys

import numpy as np

sys.path.insert(0, "/opt/trn_rl_repo")

N_CORES = 8
B_TOTAL = 16
B_LOC = B_TOTAL // N_CORES  # 2

INP_DIM = 608
NC_CLS = 80  # num classes
CH = 85  # 5 + classes
K_ROWS = 90  # 85 data + 2 raw wh + ones + gx + gy

# (H, W, anchors[(w,h)x3]) per scale; strides 8/16/32
_SCALE_DEFS = [
    (76, 76, [(10.0, 13.0), (16.0, 30.0), (33.0, 23.0)]),
    (38, 38, [(30.0, 61.0), (62.0, 45.0), (59.0, 119.0)]),
    (19, 19, [(116.0, 90.0), (156.0, 198.0), (373.0, 326.0)]),
]


def _scales():
    out = []
    off = 0
    for h, w, anchors in _SCALE_DEFS:
        hw = h * w
        stride = INP_DIM // h
        out.append(dict(H=h, W=w, HW=hw, stride=float(stride), anchors=anchors, off=off))
        off += 3 * hw
    return out, off


SCALES, N_ROWS = _scales()  # N_ROWS == 22743


def _make_weight(stride: float) -> np.ndarray:
    """[90, 85] matmul weight: transpose + sigmoid affine + grid/stride.
    All entries (0.5, 0.5*stride, stride, 1) are exact in bf16."""
    W = np.zeros((K_ROWS, CH), dtype=np.float32)
    for c in range(CH):
        if c in (0, 1):
            W[c, c] = 0.5 * stride
        elif c in (2, 3):
            pass  # raw wh comes from rows 85/86
        else:
            W[c, c] = 0.5
    W[85, 2] = 1.0
    W[86, 3] = 1.0
    # ones row: sigmoid's +0.5 (stride-scaled for x/y)
    W[87, 0] = W[87, 1] = 0.5 * stride
    W[87, 4:] = 0.5
    W[88, 0] = stride  # gx row
    W[89, 1] = stride  # gy row
    return W


def _make_gridones(h: int, w: int) -> np.ndarray:
    """[3, HW]: ones, grid_x, grid_y."""
    go = np.empty((3, h * w), dtype=np.float32)
    go[0] = 1.0
    go[1] = np.tile(np.arange(w, dtype=np.float32), h)
    go[2] = np.repeat(np.arange(h, dtype=np.float32), w)
    return go


def _patch_tile_drain():
    """The kernel-tail drain Tile emits carries one sem-wait per outstanding
    processor; this container's walrus rejects >1 sync wait on a Drain
    (CoreV3 setupSyncWait "Too many sync wait commands"). Split the waits
    across a chain of single-wait drains — same semantics, compiles."""
    import concourse.mybir as mybir
    from concourse import tile as _tile
    from concourse.vector_clock import ScopedClock

    if getattr(_tile.TileContext, "_drain_split_patched", False):
        return

    def _drain_and_barrier(self, tick_clock, wait_clock):
        drain_inst = self.nc.sync.drain()
        wait_clock.add_sem_waits(
            drain_inst.ins, ScopedClock({None: tick_clock.global_clock})
        )
        si = drain_inst.ins.sync_info
        if si is not None and len(si.on_wait) > 1:
            extra = list(si.on_wait[1:])
            del si.on_wait[1:]
            for w in extra:
                d2 = self.nc.sync.drain()
                si2 = d2.ins.sync_info
                if si2 is None:
                    d2.ins.sync_info = mybir.SyncInfo(on_wait=[w], on_update=[])
                else:
                    si2.on_wait.append(w)
        self.nc.all_engine_barrier()
        assert self.sems is not None
        popped = self.nc._tile_sem_poison_stack.pop()
        assert popped is self._sem_poison
        self.nc.clear_and_free_semaphores(list(self.sems.allocated().values()))
        self.nc.all_engine_barrier()

    _tile.TileContext._drain_and_barrier = _drain_and_barrier
    _tile.TileContext._drain_split_patched = True


_WAIT_CAP = 1


def _split_sync_waits(bir_json: bytes) -> bytes:
    """This container's walrus rejects instructions carrying more than one
    sync wait command. Move extra waits onto injected NoOps immediately
    before the instruction on the same engine queue (sequencers execute in
    order, so the combined wait semantics are identical)."""
    import json as _json

    d = _json.loads(bir_json)
    n = 0
    for f in d.get("functions", []):
        for bb in f.get("blocks", []):
            ins_list = bb.get("instructions", [])
            out = []
            for ins in ins_list:
                si = ins.get("sync_info")
                waits = (si or {}).get("on_wait") or []
                if len(waits) > _WAIT_CAP:
                    keep = waits[-_WAIT_CAP:]
                    extra = waits[: -_WAIT_CAP]
                    for i in range(0, len(extra), _WAIT_CAP):
                        n += 1
                        out.append(
                            {
                                "name": f"I-wsplit-{n}",
                                "opcode": "NoOp",
                                "engine": ins["engine"],
                                "ins": [],
                                "outs": [],
                                "bass_nofuse": True,
                                "sync_info": {
                                    "on_wait": extra[i : i + _WAIT_CAP],
                                    "on_update": [],
                                },
                            }
                        )
                    si["on_wait"] = keep
                out.append(ins)
            bb["instructions"] = out
    return _json.dumps(d).encode()


def _patch_compile():
    import concourse.bass_utils as bu

    if getattr(bu, "_wait_split_patched", False):
        return
    orig = bu.compile_bir_kernel

    def compile_bir_kernel_split(bir_json, tmpdir, neff_name="file.neff"):
        return orig(_split_sync_waits(bir_json), tmpdir, neff_name)

    bu.compile_bir_kernel = compile_bir_kernel_split
    bu._wait_split_patched = True
    import concourse.bass2jax as b2j

    b2j.compile_bir_kernel = compile_bir_kernel_split


def _build_program():
    import concourse.bass as bass
    import concourse.mybir as mybir
    from concourse.tile import TileContext

    _patch_tile_drain()
    _patch_compile()

    f32 = mybir.dt.float32
    bf16 = mybir.dt.bfloat16
    AF = mybir.ActivationFunctionType

    nc = bass.Bass()

    x_dram = [
        nc.dram_tensor(f"x{s}", [B_LOC, 255, sc["HW"]], f32, kind="ExternalInput")
        for s, sc in enumerate(SCALES)
    ]
    w_dram = [
        nc.dram_tensor(f"w{s}", [K_ROWS, CH], bf16, kind="ExternalInput")
        for s in range(3)
    ]
    go_dram = [
        nc.dram_tensor(f"go{s}", [3, SCALES[s]["HW"]], bf16, kind="ExternalInput")
        for s in range(3)
    ]
    # ln(anchor) activation biases, one column per (scale, anchor, w|h),
    # replicated down all 128 partitions.
    ln_dram = nc.dram_tensor("lnA", [128, 18], f32, kind="ExternalInput")
    out = nc.dram_tensor("out", [B_LOC, N_ROWS, CH], f32, kind="ExternalOutput")

    GROUP = 6  # transpose chunks per PSUM bank (6*85 = 510 <= 512 f32)

    with TileContext(nc) as tc:
        with (
            tc.tile_pool(name="consts", bufs=1) as cpool,
            tc.tile_pool(name="res", bufs=3) as rpool,
            tc.tile_pool(name="obuf", bufs=3) as opool,
            tc.tile_pool(name="psum", bufs=6, space="PSUM") as ppool,
        ):
            w_sb = []
            for s in range(3):
                wt = cpool.tile([K_ROWS, CH], bf16, tag=f"w{s}")
                nc.sync.dma_start(out=wt[:], in_=w_dram[s][:])
                w_sb.append(wt)
            ln_sb = cpool.tile([128, 18], f32, tag="lnA")
            nc.sync.dma_start(out=ln_sb[:], in_=ln_dram[:])

            for b in range(B_LOC):
                for s, sc in enumerate(SCALES):
                    hw = sc["HW"]
                    nfull = hw // 128
                    rem = hw - 128 * nfull
                    nchunk = nfull + (1 if rem else 0)
                    for a in range(3):
                        # raw f32 head slice; tanh casts it into the bf16
                        # matmul operand tile. Split 85 rows into 80+5: the
                        # HWDGE spreads a P-partition load over the largest
                        # divisor of P that is <=16 engines (85 -> only 5;
                        # 80 -> all 16).
                        xf = rpool.tile([85, hw], f32, tag="xf")
                        nc.sync.dma_start(
                            out=xf[0:80, :], in_=x_dram[s][b, 85 * a : 85 * a + 80, :]
                        )
                        nc.sync.dma_start(
                            out=xf[80:85, :],
                            in_=x_dram[s][b, 85 * a + 80 : 85 * a + 85, :],
                        )
                        res = rpool.tile([K_ROWS, hw], bf16, tag="res")
                        # raw w/h rows, cast f32->bf16 during DMA (SWDGE)
                        nc.gpsimd.dma_start(
                            out=res[85:87, :],
                            in_=x_dram[s][b, 85 * a + 2 : 85 * a + 4, :],
                        )
                        nc.sync.dma_start(out=res[87:90, :], in_=go_dram[s][:])

                        # sigmoid via tanh: rows 2,3 get overwritten garbage
                        # (their weight rows are zero); rows 85..89 stay raw.
                        nc.scalar.activation(
                            out=res[0:85, :], in_=xf[:], func=AF.Tanh, scale=0.5
                        )

                        obuf = opool.tile([128, nchunk * CH], f32, tag="obuf")
                        # Strided position chunks: chunk k covers positions
                        # {k + nfull*i}, so PSUM/obuf partition i accumulates
                        # nfull consecutive output rows -> the store DMA gets
                        # nfull*340B contiguous per partition.
                        res_str = (
                            res[:, 0 : nfull * 128].rearrange(
                                "p (i r) -> p r i", r=nfull
                            )
                            if nfull
                            else None
                        )
                        ngroups = math.ceil(nchunk / GROUP)
                        for g in range(ngroups):
                            k0 = g * GROUP
                            k1 = min(k0 + GROUP, nchunk)
                            psum = ppool.tile([128, GROUP * CH], f32, tag="ps")
                            for k in range(k0, k1):
                                if k < nfull:
                                    lhsT = res_str[:, k, :]
                                    m = 128
                                else:
                                    lhsT = res[:, 128 * nfull : hw]
                                    m = rem
                                nc.tensor.matmul(
                                    psum[0:m, (k - k0) * CH : (k - k0) * CH + CH],
                                    lhsT=lhsT,
                                    rhs=w_sb[s][:],
                                    start=True,
                                    stop=True,
                                )
                            # PSUM -> SBUF; avoid copying unwritten partitions
                            # of the partial chunk (stale data).
                            n_fullchunks_in_g = min(k1, nfull) - k0
                            if n_fullchunks_in_g > 0:
                                wcols = n_fullchunks_in_g * CH
                                nc.vector.tensor_copy(
                                    out=obuf[:, k0 * CH : k0 * CH + wcols],
                                    in_=psum[:, 0:wcols],
                                )
                            if k1 == nchunk and rem:
                                pcol = (nfull - k0) * CH
                                nc.vector.tensor_copy(
                                    out=obuf[0:rem, nfull * CH : nfull * CH + CH],
                                    in_=psum[0:rem, pcol : pcol + CH],
                                )

                        # w/h: exp(x + ln(anchor)) in place on strided cols 2,3
                        ob3 = obuf.rearrange("p (k c) -> p k c", c=CH)
                        for col in (2, 3):
                            j = (s * 3 + a) * 2 + (col - 2)
                            if nfull:
                                nc.scalar.activation(
                                    out=ob3[:, 0:nfull, col : col + 1],
                                    in_=ob3[:, 0:nfull, col : col + 1],
                                    func=AF.Exp,
                                    bias=ln_sb[:, j : j + 1],
                                )
                            if rem:
                                nc.scalar.activation(
                                    out=ob3[0:rem, nfull : nfull + 1, col : col + 1],
                                    in_=ob3[0:rem, nfull : nfull + 1, col : col + 1],
                                    func=AF.Exp,
                                    bias=ln_sb[0:rem, j : j + 1],
                                )

                        base = sc["off"] + a * hw
                        if nfull:
                            # partition p <-> rows [base + p*nfull, +nfull):
                            # one contiguous nfull*340B descriptor per partition
                            dst = out[b, base : base + 128 * nfull, :].rearrange(
                                "(p r) c -> p (r c)", p=128
                            )
                            nc.scalar.dma_start(out=dst, in_=obuf[:, 0 : nfull * CH])
                        if rem:
                            nc.scalar.dma_start(
                                out=out[b, base + 128 * nfull : base + hw, :],
                                in_=obuf[0:rem, nfull * CH : nfull * CH + CH],
                            )
    return nc


_PROGRAM = None
LAST_RESULT = None


def _get_program():
    global _PROGRAM
    if _PROGRAM is None:
        _PROGRAM = _build_program()
    return _PROGRAM


def kernel(x1: np.ndarray, x2: np.ndarray, x3: np.ndarray) -> np.ndarray:
    global LAST_RESULT
    from concourse.bass_utils import run_bass_kernel_spmd

    nc = _get_program()

    import ml_dtypes

    bf16 = ml_dtypes.bfloat16
    xs = [
        np.ascontiguousarray(x, dtype=np.float32).reshape(B_TOTAL, 255, sc["HW"])
        for x, sc in zip((x1, x2, x3), SCALES)
    ]
    w_consts = [_make_weight(sc["stride"]).astype(bf16) for sc in SCALES]
    go_consts = [_make_gridones(sc["H"], sc["W"]).astype(bf16) for sc in SCALES]
    ln_vals = np.array(
        [math.log(v) for sc in SCALES for anc in sc["anchors"] for v in anc],
        dtype=np.float32,
    )
    ln_const = np.broadcast_to(ln_vals, (128, 18)).copy()

    in_maps = []
    for i in range(N_CORES):
        m = {"lnA": ln_const}
        for s in range(3):
            m[f"x{s}"] = xs[s][i * B_LOC : (i + 1) * B_LOC]
            m[f"w{s}"] = w_consts[s]
            m[f"go{s}"] = go_consts[s]
        in_maps.append(m)

    LAST_RESULT = run_bass_kernel_spmd(nc, in_maps, core_ids=list(range(N_CORES)))
    return np.concatenate([r["out"] for r in LAST_RESULT.results], axis=0)
